# revision 1
# baseline (speedup 1.0000x reference)
"""Causal GQA attention (qk-norm + rope) on 8 TRN2 NeuronCores.

Sharding: tensor-parallel over heads. Core c owns Q heads {2c, 2c+1} and
KV group c//2 (w_qkv column-parallel, w_o row-parallel). Each core
computes a full-shape partial of the output projection in bf16; the host
sums the 8 partials in fp32 (row-parallel w_o => partial sums, no
on-device collective).

Per-core pipeline (all matmuls bf16 on PE, fp32 PSUM accumulate), one
merged loop over the 8 superblocks so projection/norm/rope work hides
under the ACT-bound attention stream:
  iter S: load x^T superblock (pre-transposed on host, straight 1MB DMA);
    qkv = x @ w_qkv_c in two 2-block PSUM halves; L2 qk-norm + rope
    batched on DVE in bf16 (ACT only does the sqrt); PE-transpose
    q-hat/k-hat into [hd, s]; then flash-style causal attention for
    q-chunk S: both heads interleaved per 256-row k-block pair, exp on
    ACT over [128, 2x512] (scale 1/8 folded in; qk-norm bounds scores to
    +-1/8 so no max subtraction), causal mask post-exp as 0/1 bf16
    multiply, A^T V accumulation with an appended ones column producing
    the softmax denominator for free.
  tail: y_partial = out_heads @ w_o_rows, PSUM->SBUF copies split over
    ACT+DVE, one 1MB bf16 DMA per 512-row chunk.

PSUM budget (8 banks): qkv half 1, transposes 1, sp0/sp1 2+2, av0/av1
1+1; the projection reuses the sp slots.
"""

import os

import numpy as np
import ml_dtypes

import concourse.bass as bass
import concourse.tile as tile
from concourse import bacc, mybir
from concourse.bass_utils import run_bass_kernel_spmd

F32 = mybir.dt.float32
BF16 = mybir.dt.bfloat16
AF = mybir.ActivationFunctionType

T = 4096          # sequence length
D = 1024          # d_model
HD = 64           # head dim
NB = T // 128     # 32 seq blocks of 128
NSB = T // 512    # 8 super blocks of 512
NCORES = 8
THETA = 10000.0

_built = {}


class _nullctx:
    def __enter__(self):
        return None

    def __exit__(self, *a):
        return False


def _xt_prefetch(nc, S, xstage, xbT_d):
    """Issue the 1MB x^T load for superblock S (two streams ahead of use
    so the qkv matmuls never head-of-line block the PE queue on it)."""
    xT = xstage.tile([128, 8, 512], BF16, tag="xT")
    nc.sync.dma_start(
        xT[:],
        xbT_d[:, S * 512:(S + 1) * 512].rearrange("(j p) s -> p j s", p=128))
    return xT


def _p1_main(nc, S, xT, p1w, p1ps, wqkv_b, cos_sb, sin_sb, VT):
    """qkv projection + qk-norm + rope for superblock S (everything up to
    q-hat; no PE transposes so the PE queue isn't head-of-line blocked on
    the DVE chain). Returns the qhat tile."""
    qk_s = p1w.tile([128, 4, 192], BF16, tag="qk_s")
    for half in range(2):
        qkvp = p1ps.tile([128, 2, 256], F32, tag="qkvp")
        for b2 in range(2):
            b = 2 * half + b2
            for j in range(8):
                nc.tensor.matmul(qkvp[:, b2, :],
                                 xT[:, j, b * 128:(b + 1) * 128],
                                 wqkv_b[:, j, :],
                                 start=(j == 0), stop=(j == 7))
        nc.vector.tensor_copy(VT[:, 4 * S + 2 * half:4 * S + 2 * half + 2, 0:64],
                              qkvp[:, :, 192:256])
        nc.vector.tensor_copy(qk_s[:, 2 * half:2 * half + 2, :],
                              qkvp[:, :, 0:192])

    sq = p1w.tile([128, 4, 192], BF16, tag="sq")
    ss = p1w.tile([128, 4, 3], F32, tag="ss")
    nc.vector.tensor_mul(sq[:], qk_s[:], qk_s[:])
    nc.vector.reduce_sum(ss[:], sq.rearrange("p b (h d) -> p b h d", h=3),
                         axis=mybir.AxisListType.X)
    # 1/sqrt(ss) entirely on DVE (exact reciprocal + linear seed + 2
    # Newton rsqrt iterations, multiplies only) so ACT stays on the Exp
    # table set the whole kernel (no per-superblock table reloads).
    # ss = |q|^2 ~ 0.41*chi2_64 lands in [8, 80]; seed err <~12%, two
    # iterations bring it under 1e-3.
    OPM, OPA = mybir.AluOpType.mult, mybir.AluOpType.add
    z = p1w.tile([128, 4, 3], F32, tag="z")
    nc.vector.reciprocal(z[:], ss[:])
    y = p1w.tile([128, 4, 3], F32, tag="y")
    nc.vector.tensor_scalar(y[:], z[:], 2.19, 0.098, OPM, OPA)
    t = p1w.tile([128, 4, 3], F32, tag="t")
    for _ in range(2):
        nc.vector.tensor_mul(t[:], y[:], y[:])
        nc.vector.tensor_mul(t[:], t[:], ss[:])
        nc.vector.tensor_scalar(t[:], t[:], -0.5, 1.5, OPM, OPA)
        nc.vector.tensor_mul(y[:], y[:], t[:])
    invn = p1w.tile([128, 4, 3, 1], BF16, tag="invn")
    nc.vector.tensor_copy(invn.rearrange("p b h o -> p b (h o)"), y[:])

    # batched rotate-half rope over [128, 4 blocks, 3 heads, 32]
    qv = qk_s.rearrange("p b (h d) -> p b h d", h=3)
    t1, t2 = qv[:, :, :, 0:32], qv[:, :, :, 32:64]
    cs = cos_sb[:, S].rearrange("p (b h) c -> p b h c", b=4)
    sn = sin_sb[:, S].rearrange("p (b h) c -> p b h c", b=4)
    r1 = p1w.tile([128, 4, 3, 32], BF16, tag="r1")
    r2 = p1w.tile([128, 4, 3, 32], BF16, tag="r2")
    rot = p1w.tile([128, 4, 3, 64], BF16, tag="rot")
    nc.vector.tensor_mul(r1[:], t1, cs)
    nc.vector.tensor_mul(r2[:], t2, sn)
    nc.vector.tensor_sub(rot[:, :, :, 0:32], r1[:], r2[:])
    nc.vector.tensor_mul(r1[:], t2, cs)
    nc.vector.tensor_mul(r2[:], t1, sn)
    nc.vector.tensor_add(rot[:, :, :, 32:64], r1[:], r2[:])

    # normalize (scale by 1/||.||): one DVE op via stride-0 broadcast
    qhat = p1w.tile([128, 4, 192], BF16, tag="qhat")
    qh = qhat.rearrange("p b (h d) -> p b h d", h=3)
    a_ap, b_ap = bass.broadcast_tensor_aps(rot[:, :, :, :], invn[:, :, :, :])
    nc.vector.tensor_mul(qh, a_ap, b_ap)
    return qhat


def _p1_tr(nc, S, qhat, p1pt, id_sb, QT0, QT1, KT):
    """PE-transpose q-hat / k-hat of superblock S into [hd, s]. Emitted
    late (end of the previous attention stream) so qhat is ready by the
    time these reach the PE queue head."""
    tr = p1pt.tile([128, 4, 256], BF16, tag="tr")
    for b in range(4):
        nc.tensor.transpose(tr[:, b, 0:128], qhat[:, b, 0:128], id_sb[:])
        nc.tensor.transpose(tr[0:64, b, 128:256], qhat[:, b, 128:192], id_sb[:])
    s0 = S * 512
    qt0_v = QT0[:, s0:s0 + 512].rearrange("p (b s) -> p b s", s=128)
    qt1_v = QT1[:, s0:s0 + 512].rearrange("p (b s) -> p b s", s=128)
    kt_v = KT[:, s0:s0 + 512].rearrange("p (b s) -> p b s", s=128)
    nc.vector.tensor_copy(qt0_v, tr[0:64, :, 0:128])
    nc.vector.tensor_copy(qt1_v, tr[64:128, :, 0:128])
    nc.vector.tensor_copy(kt_v, tr[0:64, :, 128:256])


LAG = 4          # av matmuls trail their scores by LAG pairs (hides exp
                 # latency behind later score matmuls in the in-order PE queue)


def _p2_qchunk(nc, qc, p2s, p2av, p2sb, p2n, QT0, QT1, KT, VT, mask_sb, OT,
               after_pair0=None, at_end=None):
    """Causal attention for 512-row q-chunk qc, both heads interleaved."""
    q0 = qc * 512
    npair = 2 * qc + 2
    av0 = p2av.tile([65, 512], F32, tag="av0")
    av1 = p2av.tile([65, 512], F32, tag="av1")
    aps = {}

    def emit_scores(p):
        sp0 = p2s.tile([128, 2, 512], F32, tag="sp0")
        sp1 = p2s.tile([128, 2, 512], F32, tag="sp1")
        for j in range(2):
            kslc = KT[:, (2 * p + j) * 128:(2 * p + j + 1) * 128]
            nc.tensor.matmul(sp0[:, j, :], kslc, QT0[:, q0:q0 + 512],
                             start=True, stop=True)
            nc.tensor.matmul(sp1[:, j, :], kslc, QT1[:, q0:q0 + 512],
                             start=True, stop=True)
        ap0 = p2sb.tile([128, 2, 512], BF16, tag="ap0")
        ap1 = p2sb.tile([128, 2, 512], BF16, tag="ap1")
        nc.scalar.activation(ap0[:], sp0[:], AF.Exp, scale=0.125)
        nc.scalar.activation(ap1[:], sp1[:], AF.Exp, scale=0.125)
        if p >= npair - 2:               # diagonal window: causal mask
            for j in range(2):
                i = 2 * (p - (npair - 2)) + j
                wm = 128 * (i + 1)       # mask is all-ones past col wm
                nc.vector.tensor_mul(ap0[:, j, 0:wm], ap0[:, j, 0:wm],
                                     mask_sb[:, i, 0:wm])
                nc.vector.tensor_mul(ap1[:, j, 0:wm], ap1[:, j, 0:wm],
                                     mask_sb[:, i, 0:wm])
        aps[p] = (ap0, ap1)

    def emit_avs(p):
        ap0, ap1 = aps.pop(p)
        for j in range(2):
            kb = 2 * p + j
            vslc = VT[:, kb, :]
            first, last = (kb == 0), (kb == 4 * qc + 3)
            nc.tensor.matmul(av0[:], vslc, ap0[:, j, :], start=first,
                             stop=last, skip_group_check=True)
            nc.tensor.matmul(av1[:], vslc, ap1[:, j, :], start=first,
                             stop=last, skip_group_check=True)

    for p in range(npair + LAG):
        if p < npair:
            emit_scores(p)
        if p == 0 and after_pair0 is not None:
            after_pair0()
        if p >= LAG:
            emit_avs(p - LAG)
    if at_end is not None:
        at_end()
    # normalize: row 64 of av is the softmax denominator
    for h, av in ((0, av0), (1, av1)):
        rec = p2n.tile([1, 512], F32, tag=f"rec{h}")
        nc.vector.reciprocal(rec[:], av[64:65, :])
        bcs = p2n.tile([64, 512], F32, tag=f"bcs{h}")
        nc.gpsimd.partition_broadcast(bcs[:], rec[:])
        nc.vector.tensor_mul(OT[64 * h:64 * h + 64, q0:q0 + 512],
                             av[0:64, :], bcs[:])


def _emit(tc, nc, xbT_d, wqkv_d, wo_d, cos_d, sin_d, mask_d, id_d, y_d):
    with (
        tc.tile_pool(name="pers", bufs=1) as pers,
        tc.tile_pool(name="xstage", bufs=4) as xstage,
    ):
        # persistent SBUF tensors
        QT0 = pers.tile([64, T], BF16)          # q-hat^T head 0
        QT1 = pers.tile([64, T], BF16)          # q-hat^T head 1
        KT = pers.tile([64, T], BF16)           # k-hat^T
        VT = pers.tile([128, NB, 65], BF16)     # per k-block [V | 1]
        OT = pers.tile([128, T], BF16)          # normalized attn out^T (2 heads)
        wqkv_b = pers.tile([128, 8, 256], BF16)
        wo_b = pers.tile([128, D], BF16)
        cos_sb = pers.tile([128, NSB, 12, 32], BF16)
        sin_sb = pers.tile([128, NSB, 12, 32], BF16)
        mask_sb = pers.tile([128, 4, 512], BF16)
        id_sb = pers.tile([128, 128], BF16)

        nc.vector.memset(VT[:, :, 64], 1.0)

        with (
            tc.tile_pool(name="p1w", bufs=4) as p1w,
            tc.tile_pool(name="p1ps", bufs=1, space="PSUM") as p1ps,
            tc.tile_pool(name="p1pt", bufs=1, space="PSUM") as p1pt,
            tc.tile_pool(name="p2s", bufs=1, space="PSUM") as p2s,
            tc.tile_pool(name="p2av", bufs=1, space="PSUM") as p2av,
            tc.tile_pool(name="p2sb", bufs=8) as p2sb,
            tc.tile_pool(name="p2ys", bufs=3) as p2ys,
            tc.tile_pool(name="p2n", bufs=2) as p2n,
            (tc.For_i(0, int(os.environ.get("K_REP", "1")), 1,
                      hint_engines=(mybir.EngineType.PE,
                                    mybir.EngineType.Activation,
                                    mybir.EngineType.DVE,
                                    mybir.EngineType.SP))
             if os.environ.get("K_REP", "1") != "1" else _nullctx()),
        ):
            # merged loop: superblock S+1's qkv/norm/rope is emitted just
            # after q-chunk S's first score pair (its DVE chain runs under
            # the attention stream); its PE transposes are emitted at the
            # end of the stream so qhat is ready when they reach the
            # in-order PE queue head.
            # emission order = first-use order: x superblock 0 and wqkv
            # gate the first matmul; wo is only needed by the proj tail
            xts = {0: _xt_prefetch(nc, 0, xstage, xbT_d)}
            nc.sync.dma_start(wqkv_b[:],
                              wqkv_d.rearrange("(j p) c -> p j c", p=128))
            xts[1] = _xt_prefetch(nc, 1, xstage, xbT_d)
            nc.sync.dma_start(cos_sb[:], cos_d[:])
            nc.sync.dma_start(sin_sb[:], sin_d[:])
            nc.sync.dma_start(id_sb[:], id_d[:])
            nc.sync.dma_start(mask_sb[:], mask_d.rearrange("i p q -> p i q"))
            nc.sync.dma_start(wo_b[:], wo_d[:])
            qhat = _p1_main(nc, 0, xts.pop(0), p1w, p1ps, wqkv_b,
                            cos_sb, sin_sb, VT)
            _p1_tr(nc, 0, qhat, p1pt, id_sb, QT0, QT1, KT)
            for S in range(NSB):
                hooks = {}
                if S + 1 < NSB:
                    def mk(vS):
                        def after_pair0():
                            if vS + 1 < NSB:
                                xts[vS + 1] = _xt_prefetch(nc, vS + 1,
                                                           xstage, xbT_d)
                            hooks["qhat"] = _p1_main(
                                nc, vS, xts.pop(vS), p1w, p1ps, wqkv_b,
                                cos_sb, sin_sb, VT)

                        def at_end():
                            _p1_tr(nc, vS, hooks["qhat"], p1pt, id_sb,
                                   QT0, QT1, KT)
                        return after_pair0, at_end
                    after_pair0, at_end = mk(S + 1)
                else:
                    after_pair0 = at_end = None
                _p2_qchunk(nc, S, p2s, p2av, p2sb, p2n, QT0, QT1, KT, VT,
                           mask_sb, OT, after_pair0, at_end)
            # output projection; PSUM slots reused from the score pool,
            # PSUM->SBUF copies split over ACT+DVE (both idle in the tail)
            for qc in range(8):
                q0 = qc * 512
                ys = p2ys.tile([128, 4, 1024], BF16, tag="ys")
                for qb in range(4):
                    ot_blk = OT[:, q0 + qb * 128:q0 + (qb + 1) * 128]
                    for nh in range(2):
                        # rotate single-bank tiles through six PSUM tag
                        # slots (all free once the attention stream drains)
                        pool, tag = ((p2s, "sp0"), (p2s, "sp1"),
                                     (p2av, "av0"), (p2av, "av1"),
                                     (p1ps, "qkvp"), (p1pt, "tr"))[
                                         (2 * qb + nh) % 6]
                        yp = pool.tile([128, 512], F32, tag=tag)
                        nc.tensor.matmul(yp[:], ot_blk,
                                         wo_b[:, nh * 512:(nh + 1) * 512],
                                         start=True, stop=True)
                        cp = (nc.scalar.copy if nh == 0
                              else nc.vector.tensor_copy)
                        cp(ys[:, qb, nh * 512:(nh + 1) * 512], yp[:])
                nc.sync.dma_start(
                    y_d[q0:q0 + 512].rearrange("(b p) d -> p b d", p=128),
                    ys[:])


def _build():
    key = os.environ.get("K_REP", "1")
    if key in _built:
        return _built[key]
    nc = bacc.Bacc("TRN2", target_bir_lowering=False, debug=False)
    xbT_d = nc.dram_tensor("xbT", [D, T], BF16, kind="ExternalInput").ap()
    wqkv_d = nc.dram_tensor("wqkv", [D, 256], BF16, kind="ExternalInput").ap()
    wo_d = nc.dram_tensor("wo", [128, D], BF16, kind="ExternalInput").ap()
    cos_d = nc.dram_tensor("cos12", [128, NSB, 12, 32], BF16,
                           kind="ExternalInput").ap()
    sin_d = nc.dram_tensor("sin12", [128, NSB, 12, 32], BF16,
                           kind="ExternalInput").ap()
    mask_d = nc.dram_tensor("mask", [4, 128, 512], BF16, kind="ExternalInput").ap()
    id_d = nc.dram_tensor("ident", [128, 128], BF16, kind="ExternalInput").ap()
    y_d = nc.dram_tensor("y", [T, D], BF16, kind="ExternalOutput").ap()
    with tile.TileContext(nc) as tc:
        _emit(tc, nc, xbT_d, wqkv_d, wo_d, cos_d, sin_d, mask_d, id_d, y_d)
    nc.compile()
    _built[key] = nc
    return nc


def _static_tables():
    half = HD // 2
    inv_freq = 1.0 / (THETA ** (np.arange(half, dtype=np.float32) / half))
    ang = np.arange(T, dtype=np.float32)[:, None] * inv_freq[None, :]

    # [T, 32] -> [128 partition, NSB, 4 blocks, 3 heads, 32] -> flatten b,h
    def tab12(f):
        t = f(ang).astype(np.float32).reshape(NSB, 4, 128, half)
        t = np.transpose(t, (2, 0, 1, 3))              # [128, NSB, 4, 32]
        t = np.repeat(t[:, :, :, None, :], 3, axis=3)  # [128, NSB, 4, 3, 32]
        return np.ascontiguousarray(
            t.reshape(128, NSB, 12, 32).astype(ml_dtypes.bfloat16))
    cos12 = tab12(np.cos)
    sin12 = tab12(np.sin)

    kl = np.arange(128)[None, :, None]
    ql = np.arange(512)[None, None, :]
    iv = np.arange(4)[:, None, None]
    mask = (ql >= kl + 128 * iv).astype(ml_dtypes.bfloat16)
    ident = np.eye(128, dtype=ml_dtypes.bfloat16)
    return cos12, sin12, mask, ident


_tables = None


def host_inputs(x, w_qkv, w_o):
    """Per-core input dicts (shards + constant tables)."""
    global _tables
    if _tables is None:
        _tables = _static_tables()
    cos12, sin12, mask, ident = _tables

    x2 = np.asarray(x, np.float32).reshape(T, D).astype(ml_dtypes.bfloat16)
    xbT = np.ascontiguousarray(x2.T)                   # [D, T] bf16
    w_qkv = np.asarray(w_qkv, np.float32)
    w_o = np.asarray(w_o, np.float32)

    maps = []
    for c in range(NCORES):
        g = c // 2
        wq = np.ascontiguousarray(np.concatenate([
            w_qkv[:, 128 * c:128 * c + 128],              # 2 q heads
            w_qkv[:, 1024 + 64 * g:1024 + 64 * g + 64],   # k group
            w_qkv[:, 1280 + 64 * g:1280 + 64 * g + 64],   # v group
        ], axis=1).astype(ml_dtypes.bfloat16))
        wo_c = np.ascontiguousarray(
            w_o[128 * c:128 * c + 128, :].astype(ml_dtypes.bfloat16))
        maps.append(dict(xbT=xbT, wqkv=wq, wo=wo_c, cos12=cos12, sin12=sin12,
                         mask=mask, ident=ident))
    return maps


def kernel(x, w_qkv, w_o):
    nc = _build()
    maps = host_inputs(x, w_qkv, w_o)
    res = run_bass_kernel_spmd(nc, maps, list(range(NCORES))).results
    y = np.zeros((T, D), np.float32)
    for c in range(NCORES):
        y += np.asarray(res[c]["y"]).astype(np.float32)
    return y.reshape(1, T, D)



# revision 6
# speedup vs baseline: 1.1457x; 1.1457x over previous
"""Causal GQA attention (qk-norm + rope) on 8 TRN2 NeuronCores.

Sharding: tensor-parallel over heads. Core c owns Q heads {2c, 2c+1} and
KV group c//2 (w_qkv column-parallel, w_o row-parallel). Each core
computes a full-shape partial of the output projection in bf16; the host
sums the 8 partials in fp32 (row-parallel w_o => partial sums, no
on-device collective).

Per-core pipeline (all matmuls bf16 on PE, fp32 PSUM accumulate), one
merged loop over the 8 superblocks so projection/norm/rope work hides
under the ACT-bound attention stream:
  iter S: load x^T superblock (pre-transposed on host, straight 1MB DMA);
    qkv = x @ w_qkv_c in two 2-block PSUM halves; L2 qk-norm + rope
    batched on DVE in bf16 (ACT only does the sqrt); PE-transpose
    q-hat/k-hat into [hd, s]; then flash-style causal attention for
    q-chunk S: both heads interleaved per 256-row k-block pair, exp on
    ACT over [128, 2x512] (scale 1/8 folded in; qk-norm bounds scores to
    +-1/8 so no max subtraction), causal mask post-exp as 0/1 bf16
    multiply, A^T V accumulation with an appended ones column producing
    the softmax denominator for free.
  tail: y_partial = out_heads @ w_o_rows, PSUM->SBUF copies split over
    ACT+DVE, one 1MB bf16 DMA per 512-row chunk.

PSUM budget (8 banks): qkv half 1, transposes 1, sp0/sp1 2+2, av0/av1
1+1; the projection reuses the sp slots.
"""

import os

import numpy as np
import ml_dtypes

import concourse.bass as bass
import concourse.tile as tile
from concourse import bacc, mybir
from concourse.bass_utils import run_bass_kernel_spmd

F32 = mybir.dt.float32
BF16 = mybir.dt.bfloat16
AF = mybir.ActivationFunctionType

T = 4096          # sequence length
D = 1024          # d_model
HD = 64           # head dim
NB = T // 128     # 32 seq blocks of 128
NSB = T // 512    # 8 super blocks of 512
NCORES = 8
THETA = 10000.0

_built = {}


def _abl():
    """Timing-only ablation mode (never set by the grading harness):
    '' full kernel | 'noav' skip AV matmuls+normalize | 'noexp' skip
    exp/mask, AV reads a dummy ap | 'noattn' skip the whole attention
    stream | 'noproj' skip qkv/rope/transposes, attention reads memset
    QT/KT/VT."""
    return os.environ.get("K_ABL", "")


class _nullctx:
    def __enter__(self):
        return None

    def __exit__(self, *a):
        return False


def _xt_prefetch(nc, S, xstage, xbT_d):
    """Issue the 1MB x^T load for superblock S (two streams ahead of use
    so the qkv matmuls never head-of-line block the PE queue on it)."""
    xT = xstage.tile([128, 8, 512], BF16, tag="xT")
    nc.sync.dma_start(
        xT[:],
        xbT_d[:, S * 512:(S + 1) * 512].rearrange("(j p) s -> p j s", p=128))
    return xT


def _p1_main(nc, S, xT, p1w, p1ps, wqkv_b, cos_sb, sin_sb, VT):
    """qkv projection + qk-norm + rope for superblock S (everything up to
    q-hat; no PE transposes so the PE queue isn't head-of-line blocked on
    the DVE chain). Returns the qhat tile."""
    qk_s = p1w.tile([128, 4, 192], BF16, tag="qk_s")
    for half in range(2):
        qkvp = p1ps.tile([128, 2, 256], F32, tag="qkvp")
        for b2 in range(2):
            b = 2 * half + b2
            for j in range(8):
                nc.tensor.matmul(qkvp[:, b2, :],
                                 xT[:, j, b * 128:(b + 1) * 128],
                                 wqkv_b[:, j, :],
                                 start=(j == 0), stop=(j == 7))
        nc.vector.tensor_copy(VT[:, 4 * S + 2 * half:4 * S + 2 * half + 2, 0:64],
                              qkvp[:, :, 192:256])
        nc.vector.tensor_copy(qk_s[:, 2 * half:2 * half + 2, :],
                              qkvp[:, :, 0:192])

    sq = p1w.tile([128, 4, 192], BF16, tag="sq")
    ss = p1w.tile([128, 4, 3], F32, tag="ss")
    nc.vector.tensor_mul(sq[:], qk_s[:], qk_s[:])
    nc.vector.reduce_sum(ss[:], sq.rearrange("p b (h d) -> p b h d", h=3),
                         axis=mybir.AxisListType.X)
    # 1/sqrt(ss) entirely on DVE (exact reciprocal + linear seed + 2
    # Newton rsqrt iterations, multiplies only) so ACT stays on the Exp
    # table set the whole kernel (no per-superblock table reloads).
    # ss = |q|^2 ~ 0.41*chi2_64 lands in [8, 80]; seed err <~12%, two
    # iterations bring it under 1e-3.
    OPM, OPA = mybir.AluOpType.mult, mybir.AluOpType.add
    z = p1w.tile([128, 4, 3], F32, tag="z")
    nc.vector.reciprocal(z[:], ss[:])
    y = p1w.tile([128, 4, 3], F32, tag="y")
    nc.vector.tensor_scalar(y[:], z[:], 2.19, 0.098, OPM, OPA)
    t = p1w.tile([128, 4, 3], F32, tag="t")
    for _ in range(2):
        nc.vector.tensor_mul(t[:], y[:], y[:])
        nc.vector.tensor_mul(t[:], t[:], ss[:])
        nc.vector.tensor_scalar(t[:], t[:], -0.5, 1.5, OPM, OPA)
        nc.vector.tensor_mul(y[:], y[:], t[:])
    invn = p1w.tile([128, 4, 3, 1], BF16, tag="invn")
    nc.vector.tensor_copy(invn.rearrange("p b h o -> p b (h o)"), y[:])

    # batched rotate-half rope over [128, 4 blocks, 3 heads, 32]
    qv = qk_s.rearrange("p b (h d) -> p b h d", h=3)
    t1, t2 = qv[:, :, :, 0:32], qv[:, :, :, 32:64]
    cs = cos_sb[:, S].rearrange("p (b h) c -> p b h c", b=4)
    sn = sin_sb[:, S].rearrange("p (b h) c -> p b h c", b=4)
    r1 = p1w.tile([128, 4, 3, 32], BF16, tag="r1")
    r2 = p1w.tile([128, 4, 3, 32], BF16, tag="r2")
    rot = p1w.tile([128, 4, 3, 64], BF16, tag="rot")
    nc.vector.tensor_mul(r1[:], t1, cs)
    nc.vector.tensor_mul(r2[:], t2, sn)
    nc.vector.tensor_sub(rot[:, :, :, 0:32], r1[:], r2[:])
    nc.vector.tensor_mul(r1[:], t2, cs)
    nc.vector.tensor_mul(r2[:], t1, sn)
    nc.vector.tensor_add(rot[:, :, :, 32:64], r1[:], r2[:])

    # normalize (scale by 1/||.||): one DVE op via stride-0 broadcast
    qhat = p1w.tile([128, 4, 192], BF16, tag="qhat")
    qh = qhat.rearrange("p b (h d) -> p b h d", h=3)
    a_ap, b_ap = bass.broadcast_tensor_aps(rot[:, :, :, :], invn[:, :, :, :])
    nc.vector.tensor_mul(qh, a_ap, b_ap)
    return qhat


def _p1_tr(nc, S, qhat, p1pt, id_sb, QT0, QT1, KT):
    """PE-transpose q-hat / k-hat of superblock S into [hd, s]. Emitted
    late (end of the previous attention stream) so qhat is ready by the
    time these reach the PE queue head."""
    tr = p1pt.tile([128, 4, 256], BF16, tag="tr")
    for b in range(4):
        nc.tensor.transpose(tr[:, b, 0:128], qhat[:, b, 0:128], id_sb[:])
        nc.tensor.transpose(tr[0:64, b, 128:256], qhat[:, b, 128:192], id_sb[:])
    s0 = S * 512
    qt0_v = QT0[:, s0:s0 + 512].rearrange("p (b s) -> p b s", s=128)
    qt1_v = QT1[:, s0:s0 + 512].rearrange("p (b s) -> p b s", s=128)
    kt_v = KT[:, s0:s0 + 512].rearrange("p (b s) -> p b s", s=128)
    nc.vector.tensor_copy(qt0_v, tr[0:64, :, 0:128])
    nc.vector.tensor_copy(qt1_v, tr[64:128, :, 0:128])
    nc.vector.tensor_copy(kt_v, tr[0:64, :, 128:256])


LAG = 4          # av matmuls trail their scores by LAG pairs (hides exp
                 # latency behind later score matmuls in the in-order PE queue)


def _p2_qchunk(nc, qc, p2s, p2av, p2sb, p2n, QT0, QT1, KT, VT, mask_sb, OT,
               after_pair0=None, at_end=None, apz=None):
    """Causal attention for 512-row q-chunk qc, both heads interleaved."""
    abl = _abl()
    if abl == "noattn":
        if after_pair0 is not None:
            after_pair0()
        if at_end is not None:
            at_end()
        return
    q0 = qc * 512
    npair = 2 * qc + 2
    av0 = p2av.tile([65, 512], F32, tag="av0")
    av1 = p2av.tile([65, 512], F32, tag="av1")
    aps = {}

    def emit_scores(p):
        sp0 = p2s.tile([128, 2, 512], F32, tag="sp0")
        sp1 = p2s.tile([128, 2, 512], F32, tag="sp1")
        for j in range(2):
            kslc = KT[:, (2 * p + j) * 128:(2 * p + j + 1) * 128]
            nc.tensor.matmul(sp0[:, j, :], kslc, QT0[:, q0:q0 + 512],
                             start=True, stop=True)
            nc.tensor.matmul(sp1[:, j, :], kslc, QT1[:, q0:q0 + 512],
                             start=True, stop=True)
        if abl == "noexp":
            aps[p] = (apz, apz)
            return
        ap0 = p2sb.tile([128, 2, 512], BF16, tag="ap0")
        ap1 = p2sb.tile([128, 2, 512], BF16, tag="ap1")
        nc.scalar.activation(ap0[:], sp0[:], AF.Exp, scale=0.125)
        nc.scalar.activation(ap1[:], sp1[:], AF.Exp, scale=0.125)
        if p >= npair - 2:               # diagonal window: causal mask
            for j in range(2):
                i = 2 * (p - (npair - 2)) + j
                wm = 128 * (i + 1)       # mask is all-ones past col wm
                nc.vector.tensor_mul(ap0[:, j, 0:wm], ap0[:, j, 0:wm],
                                     mask_sb[:, i, 0:wm])
                nc.vector.tensor_mul(ap1[:, j, 0:wm], ap1[:, j, 0:wm],
                                     mask_sb[:, i, 0:wm])
        aps[p] = (ap0, ap1)

    def emit_avs(p):
        ap0, ap1 = aps.pop(p)
        if abl == "noav":
            return
        for j in range(2):
            kb = 2 * p + j
            vslc = VT[:, kb, :]
            first, last = (kb == 0), (kb == 4 * qc + 3)
            nc.tensor.matmul(av0[:], vslc, ap0[:, j, :], start=first,
                             stop=last, skip_group_check=True)
            nc.tensor.matmul(av1[:], vslc, ap1[:, j, :], start=first,
                             stop=last, skip_group_check=True)

    for p in range(npair + LAG):
        if p < npair:
            emit_scores(p)
        if p == 0 and after_pair0 is not None:
            after_pair0()
        if p >= LAG:
            emit_avs(p - LAG)
    if at_end is not None:
        at_end()
    if abl == "noav":
        return
    # normalize: row 64 of av is the softmax denominator
    for h, av in ((0, av0), (1, av1)):
        rec = p2n.tile([1, 512], F32, tag=f"rec{h}")
        nc.vector.reciprocal(rec[:], av[64:65, :])
        bcs = p2n.tile([64, 512], F32, tag=f"bcs{h}")
        nc.gpsimd.partition_broadcast(bcs[:], rec[:])
        nc.vector.tensor_mul(OT[64 * h:64 * h + 64, q0:q0 + 512],
                             av[0:64, :], bcs[:])


def _emit(tc, nc, xbT_d, wqkv_d, wo_d, cos_d, sin_d, mask_d, id_d, y_d):
    with (
        tc.tile_pool(name="pers", bufs=1) as pers,
        tc.tile_pool(name="xstage", bufs=4) as xstage,
    ):
        # persistent SBUF tensors
        QT0 = pers.tile([64, T], BF16)          # q-hat^T head 0
        QT1 = pers.tile([64, T], BF16)          # q-hat^T head 1
        KT = pers.tile([64, T], BF16)           # k-hat^T
        VT = pers.tile([128, NB, 65], BF16)     # per k-block [V | 1]
        OT = pers.tile([128, T], BF16)          # normalized attn out^T (2 heads)
        wqkv_b = pers.tile([128, 8, 256], BF16)
        wo_b = pers.tile([128, D], BF16)
        cos_sb = pers.tile([128, NSB, 12, 32], BF16)
        sin_sb = pers.tile([128, NSB, 12, 32], BF16)
        mask_sb = pers.tile([128, 4, 512], BF16)
        id_sb = pers.tile([128, 128], BF16)

        nc.vector.memset(VT[:, :, 64], 1.0)

        abl = _abl()
        apz = None
        if abl in ("noav", "noattn"):
            nc.vector.memset(OT[:], 0.01)
        if abl == "noexp":
            apz = pers.tile([128, 2, 512], BF16)
            nc.vector.memset(apz[:], 0.5)
        if abl == "noproj":
            nc.vector.memset(QT0[:], 0.05)
            nc.vector.memset(QT1[:], 0.05)
            nc.vector.memset(KT[:], 0.05)
            nc.vector.memset(VT[:, :, 0:64], 0.05)

        with (
            tc.tile_pool(name="p1w", bufs=4) as p1w,
            tc.tile_pool(name="p1ps", bufs=1, space="PSUM") as p1ps,
            tc.tile_pool(name="p1pt", bufs=1, space="PSUM") as p1pt,
            tc.tile_pool(name="p2s", bufs=1, space="PSUM") as p2s,
            tc.tile_pool(name="p2av", bufs=1, space="PSUM") as p2av,
            tc.tile_pool(name="p2sb", bufs=8) as p2sb,
            tc.tile_pool(name="p2ys", bufs=3) as p2ys,
            tc.tile_pool(name="p2n", bufs=2) as p2n,
            (tc.For_i(0, int(os.environ.get("K_REP", "1")), 1,
                      hint_engines=(mybir.EngineType.PE,
                                    mybir.EngineType.Activation,
                                    mybir.EngineType.DVE,
                                    mybir.EngineType.SP))
             if os.environ.get("K_REP", "1") != "1" else _nullctx()),
        ):
            # merged loop: superblock S+1's qkv/norm/rope is emitted just
            # after q-chunk S's first score pair (its DVE chain runs under
            # the attention stream); its PE transposes are emitted at the
            # end of the stream so qhat is ready when they reach the
            # in-order PE queue head.
            # emission order = first-use order: x superblock 0 and wqkv
            # gate the first matmul; wo is only needed by the proj tail
            noproj = (abl == "noproj")
            if not noproj:
                xts = {0: _xt_prefetch(nc, 0, xstage, xbT_d)}
                nc.sync.dma_start(wqkv_b[:],
                                  wqkv_d.rearrange("(j p) c -> p j c", p=128))
                xts[1] = _xt_prefetch(nc, 1, xstage, xbT_d)
            nc.sync.dma_start(cos_sb[:], cos_d[:])
            nc.sync.dma_start(sin_sb[:], sin_d[:])
            nc.sync.dma_start(id_sb[:], id_d[:])
            nc.sync.dma_start(mask_sb[:], mask_d.rearrange("i p q -> p i q"))
            nc.sync.dma_start(wo_b[:], wo_d[:])
            if not noproj:
                qhat = _p1_main(nc, 0, xts.pop(0), p1w, p1ps, wqkv_b,
                                cos_sb, sin_sb, VT)
                _p1_tr(nc, 0, qhat, p1pt, id_sb, QT0, QT1, KT)
            for S in range(NSB):
                hooks = {}
                if S + 1 < NSB and not noproj:
                    def mk(vS):
                        def after_pair0():
                            if vS + 1 < NSB:
                                xts[vS + 1] = _xt_prefetch(nc, vS + 1,
                                                           xstage, xbT_d)
                            hooks["qhat"] = _p1_main(
                                nc, vS, xts.pop(vS), p1w, p1ps, wqkv_b,
                                cos_sb, sin_sb, VT)

                        def at_end():
                            _p1_tr(nc, vS, hooks["qhat"], p1pt, id_sb,
                                   QT0, QT1, KT)
                        return after_pair0, at_end
                    after_pair0, at_end = mk(S + 1)
                else:
                    after_pair0 = at_end = None
                _p2_qchunk(nc, S, p2s, p2av, p2sb, p2n, QT0, QT1, KT, VT,
                           mask_sb, OT, after_pair0, at_end, apz)
            # output projection; PSUM slots reused from the score pool,
            # PSUM->SBUF copies split over ACT+DVE (both idle in the tail)
            for qc in range(8):
                q0 = qc * 512
                ys = p2ys.tile([128, 4, 1024], BF16, tag="ys")
                for qb in range(4):
                    ot_blk = OT[:, q0 + qb * 128:q0 + (qb + 1) * 128]
                    for nh in range(2):
                        # rotate single-bank tiles through six PSUM tag
                        # slots (all free once the attention stream drains)
                        pool, tag = ((p2s, "sp0"), (p2s, "sp1"),
                                     (p2av, "av0"), (p2av, "av1"),
                                     (p1ps, "qkvp"), (p1pt, "tr"))[
                                         (2 * qb + nh) % 6]
                        yp = pool.tile([128, 512], F32, tag=tag)
                        nc.tensor.matmul(yp[:], ot_blk,
                                         wo_b[:, nh * 512:(nh + 1) * 512],
                                         start=True, stop=True)
                        cp = (nc.scalar.copy if nh == 0
                              else nc.vector.tensor_copy)
                        cp(ys[:, qb, nh * 512:(nh + 1) * 512], yp[:])
                nc.sync.dma_start(
                    y_d[q0:q0 + 512].rearrange("(b p) d -> p b d", p=128),
                    ys[:])


def _build():
    key = (os.environ.get("K_REP", "1"), _abl())
    if key in _built:
        return _built[key]
    nc = bacc.Bacc("TRN2", target_bir_lowering=False, debug=False)
    xbT_d = nc.dram_tensor("xbT", [D, T], BF16, kind="ExternalInput").ap()
    wqkv_d = nc.dram_tensor("wqkv", [D, 256], BF16, kind="ExternalInput").ap()
    wo_d = nc.dram_tensor("wo", [128, D], BF16, kind="ExternalInput").ap()
    cos_d = nc.dram_tensor("cos12", [128, NSB, 12, 32], BF16,
                           kind="ExternalInput").ap()
    sin_d = nc.dram_tensor("sin12", [128, NSB, 12, 32], BF16,
                           kind="ExternalInput").ap()
    mask_d = nc.dram_tensor("mask", [4, 128, 512], BF16, kind="ExternalInput").ap()
    id_d = nc.dram_tensor("ident", [128, 128], BF16, kind="ExternalInput").ap()
    y_d = nc.dram_tensor("y", [T, D], BF16, kind="ExternalOutput").ap()
    with tile.TileContext(nc) as tc:
        _emit(tc, nc, xbT_d, wqkv_d, wo_d, cos_d, sin_d, mask_d, id_d, y_d)
    nc.compile()
    _built[key] = nc
    return nc


def _static_tables():
    half = HD // 2
    inv_freq = 1.0 / (THETA ** (np.arange(half, dtype=np.float32) / half))
    ang = np.arange(T, dtype=np.float32)[:, None] * inv_freq[None, :]

    # [T, 32] -> [128 partition, NSB, 4 blocks, 3 heads, 32] -> flatten b,h
    def tab12(f):
        t = f(ang).astype(np.float32).reshape(NSB, 4, 128, half)
        t = np.transpose(t, (2, 0, 1, 3))              # [128, NSB, 4, 32]
        t = np.repeat(t[:, :, :, None, :], 3, axis=3)  # [128, NSB, 4, 3, 32]
        return np.ascontiguousarray(
            t.reshape(128, NSB, 12, 32).astype(ml_dtypes.bfloat16))
    cos12 = tab12(np.cos)
    sin12 = tab12(np.sin)

    kl = np.arange(128)[None, :, None]
    ql = np.arange(512)[None, None, :]
    iv = np.arange(4)[:, None, None]
    mask = (ql >= kl + 128 * iv).astype(ml_dtypes.bfloat16)
    ident = np.eye(128, dtype=ml_dtypes.bfloat16)
    return cos12, sin12, mask, ident


_tables = None


def host_inputs(x, w_qkv, w_o):
    """Per-core input dicts (shards + constant tables)."""
    global _tables
    if _tables is None:
        _tables = _static_tables()
    cos12, sin12, mask, ident = _tables

    x2 = np.asarray(x, np.float32).reshape(T, D).astype(ml_dtypes.bfloat16)
    xbT = np.ascontiguousarray(x2.T)                   # [D, T] bf16
    w_qkv = np.asarray(w_qkv, np.float32)
    w_o = np.asarray(w_o, np.float32)

    maps = []
    for c in range(NCORES):
        g = c // 2
        wq = np.ascontiguousarray(np.concatenate([
            w_qkv[:, 128 * c:128 * c + 128],              # 2 q heads
            w_qkv[:, 1024 + 64 * g:1024 + 64 * g + 64],   # k group
            w_qkv[:, 1280 + 64 * g:1280 + 64 * g + 64],   # v group
        ], axis=1).astype(ml_dtypes.bfloat16))
        wo_c = np.ascontiguousarray(
            w_o[128 * c:128 * c + 128, :].astype(ml_dtypes.bfloat16))
        maps.append(dict(xbT=xbT, wqkv=wq, wo=wo_c, cos12=cos12, sin12=sin12,
                         mask=mask, ident=ident))
    return maps


def kernel(x, w_qkv, w_o):
    nc = _build()
    maps = host_inputs(x, w_qkv, w_o)
    res = run_bass_kernel_spmd(nc, maps, list(range(NCORES))).results
    y = np.zeros((T, D), np.float32)
    for c in range(NCORES):
        y += np.asarray(res[c]["y"]).astype(np.float32)
    return y.reshape(1, T, D)

